# revision 43
# baseline (speedup 1.0000x reference)
"""Linear attention (elu(x)+1 feature map) Bass/Tile kernel for Trainium2.

Problem: B=4, H=16, S=4096, D=64, fp32.
  Qf = elu(Q)+1; Kf = (elu(K)+1)*mask
  KV = einsum('bhsd,bhse->bhde', Kf, V); Ksum = sum_s Kf
  out = (Qf @ KV) / (Qf . Ksum)

Sharding: the 64 (b,h) pairs are data-parallel; each of the 8 cores gets 8
pairs. No collectives.

Per-core design (v3; ~4x faster than the DMA-dispatch-bound v1):
  * "Fat-row" layout: s = 32*p + r (partition p holds 32 consecutive rows),
    so each pair's Q/K/V/O moves as ONE DMA of [128, 2048] with 8KB
    contiguous lines (vs 48 DMAs with 256B lines per pair before). ~29
    DMAs total; HBM transfer is the roofline (~34MB at ~360-400GB/s).
  * bf16 matmul operands (harness tolerance 2e-2; measured ~1.8e-3):
    PE runs at 1 cycle/row instead of 4 for fp32.
  * elu(x)+1 = min(exp(x),1) + relu(x) in 3 passes: exp + relu on ACT
    (bf16 out), then one fused DVE scalar_tensor_tensor
    (min(e,1) add r). GPSIMD tensor_scalar proved ~10x slow on real HW
    (software op), so relu stays on ACT.
  * mask folded into V during its bf16 conversion on Pool (V*m), and the
    raw mask appended as column 64 of the vm tile so the 130-wide KV
    accumulation matmul also yields Ksum = Kf^T m for free (both pairs
    of a group merged into one [128,130] matmul per 128-row step).
  * Qf^T via PE transposes batched 8-wide into one full PSUM bank,
    copied to SBUF alternately by ACT (Copy activation) and DVE.
  * Phase B: per r one [128,130] matmul against the block-diagonal
    [[KV_A|KsumA],0;0,[KV_B|KsumB]] gives out and denominator together;
    batches of 3 r per PSUM bank, then DVE reciprocal of the den columns
    (PSUM->SBUF; HW allows only one PSUM operand per DVE op) and one DVE
    multiply writes normalized output into the staging tile.
  * Software-pipelined across the 4 groups: group g-1's tail issues
    before group g's PE section; output DMAs go on the ACT queue (SP
    keeps the input stream), split in thirds to shorten the drain.

Benchmarked via reps-slope (K kernel bodies in one NEFF, dispatch
overhead cancels; pooled median of 24 interleaved samples): ~79us/exec
vs ~396us for the v1 baseline (~5x) — at the HBM roofline (25.2MB read
+ 8.4MB write per core, with reads/writes partially overlapping on HW).
"""

import numpy as np

import concourse.bass as bass
import concourse.mybir as mybir
import concourse.tile as tile
from concourse.bass_utils import run_bass_kernel_spmd
from concourse.masks import make_identity

F32 = mybir.dt.float32
BF16 = mybir.dt.bfloat16
AF = mybir.ActivationFunctionType
ALU = mybir.AluOpType

N_CORES = 8
PAIRS = 8          # (b,h) pairs per core
S = 4096
D = 64
R = 32             # rows per partition; s = 32*p + r
NGROUPS = PAIRS // 2


def build_bass(reps: int = 1) -> bass.Bass:
    from contextlib import ExitStack

    from concourse.bacc import Bacc
    nc = Bacc()
    Qh = nc.dram_tensor("Q", [PAIRS, S, D], F32, kind="ExternalInput")
    Kh = nc.dram_tensor("K", [PAIRS, S, D], F32, kind="ExternalInput")
    Vh = nc.dram_tensor("V", [PAIRS, S, D], F32, kind="ExternalInput")
    Mh = nc.dram_tensor("mask", [PAIRS, S], F32, kind="ExternalInput")
    Oh = nc.dram_tensor("O", [PAIRS, S, D], F32, kind="ExternalOutput")

    # per-pair fat-row views [128, 2048]; per-group output views [128,2,2048]
    def pview(h, p):
        return h[p].rearrange("(q r) d -> q (r d)", q=128, r=R)

    Mv = Mh.rearrange("u (q r) -> q u r", q=128, r=R)          # [128, 8, 32]
    Ov = [Oh[2 * g:2 * g + 2].rearrange("u (q r) d -> q u (r d)", q=128, r=R)
          for g in range(NGROUPS)]

    with tile.TileContext(nc) as tc:
        with ExitStack() as ctx:
            consts = ctx.enter_context(tc.tile_pool(name="consts", bufs=1))
            qraw_p = ctx.enter_context(tc.tile_pool(name="qraw", bufs=3))
            kraw_p = ctx.enter_context(tc.tile_pool(name="kraw", bufs=3))
            vraw_p = ctx.enter_context(tc.tile_pool(name="vraw", bufs=3))
            qe_p = ctx.enter_context(tc.tile_pool(name="qe", bufs=2))
            ke_p = ctx.enter_context(tc.tile_pool(name="ke", bufs=2))
            qr2_p = ctx.enter_context(tc.tile_pool(name="qr2", bufs=2))
            kr2_p = ctx.enter_context(tc.tile_pool(name="kr2", bufs=2))
            qf_p = ctx.enter_context(tc.tile_pool(name="qf", bufs=2))
            kf_p = ctx.enter_context(tc.tile_pool(name="kf", bufs=2))
            vm_p = ctx.enter_context(tc.tile_pool(name="vm", bufs=2))
            qt_p = ctx.enter_context(tc.tile_pool(name="qt", bufs=2))
            osb_p = ctx.enter_context(tc.tile_pool(name="osb", bufs=2))
            bd_p = ctx.enter_context(tc.tile_pool(name="bd", bufs=2))
            rec_p = ctx.enter_context(tc.tile_pool(name="rec", bufs=2))
            kv_ps = ctx.enter_context(
                tc.tile_pool(name="kvps", bufs=2, space="PSUM"))
            tp_ps = ctx.enter_context(
                tc.tile_pool(name="tpps", bufs=2, space="PSUM"))
            ob_ps = ctx.enter_context(
                tc.tile_pool(name="obps", bufs=4, space="PSUM"))

            identity = consts.tile([128, 128], BF16)
            make_identity(nc, identity)
            mtile = consts.tile([128, PAIRS, R], F32)
            nc.sync.dma_start(out=mtile, in_=Mv)

            def tail(g, kv, qt, last=False):
                g = g % NGROUPS
                """phaseB (out + den cols) / recip / normalize / output."""
                # bd = [[KV_A|KsumA], 0; 0, [KV_B|KsumB]] — one copy per pair
                bd = bd_p.tile([128, 2, D + 1], BF16)
                nc.vector.memset(bd, 0.0)
                nc.vector.tensor_copy(out=bd[0:64, 0, :], in_=kv[0:64, 0, :])
                nc.vector.tensor_copy(out=bd[64:128, 1, :],
                                      in_=kv[64:128, 1, :])

                osb = osb_p.tile([128, 2, R, D], F32)
                rec = rec_p.tile([128, R, 2], F32)
                # 32 r-steps in batches of 3 per PSUM bank (3*130 <= 512 f32)
                batches = [(s, min(3, R - s)) for s in range(0, R, 3)]
                # output DMA chunk boundaries (batch idx -> r range end)
                cuts = [2, 5, 8] if last else [4, 8]
                prev_r = [0]
                for bi, (r0, bsz) in enumerate(batches):
                    ob = ob_ps.tile([128, 3, 2, D + 1], F32)
                    for j in range(bsz):
                        nc.tensor.matmul(ob[:, j], lhsT=qt[:, r0 + j, :],
                                         rhs=bd, start=True, stop=True)
                    nc.vector.reciprocal(
                        rec[:, r0:r0 + bsz, :], ob[:, 0:bsz, :, D])
                    nc.vector.tensor_tensor(
                        out=osb[:, :, r0:r0 + bsz, :],
                        in0=ob[:, 0:bsz, :, 0:D].rearrange(
                            "q j u d -> q u j d"),
                        in1=rec[:, r0:r0 + bsz, :]
                        .rearrange("q j u -> q u j").unsqueeze(-1)
                        .to_broadcast([128, 2, bsz, D]),
                        op=ALU.mult)
                    if bi + 1 in cuts:
                        a, b2 = prev_r[0], r0 + bsz
                        nc.scalar.dma_start(
                            out=Ov[g][:, :, a * D:b2 * D],
                            in_=osb[:, :, a:b2, :].rearrange(
                                "q u r d -> q u (r d)"))
                        prev_r[0] = b2
                a = prev_r[0]
                nc.scalar.dma_start(
                    out=Ov[g][:, :, a * D:],
                    in_=osb[:, :, a:, :].rearrange("q u r d -> q u (r d)"))

            prev = None
            for p in range(PAIRS * reps):
                g, u = divmod(p % PAIRS, 2)
                g += (p // PAIRS) * NGROUPS
                p = p % PAIRS
                if u == 0:
                    kv = kv_ps.tile([128, 2, D + 1], F32)
                    qt = qt_p.tile([128, R, 128], BF16)
                    qf = qf_p.tile([128, R, 2, D], BF16)
                    kf = kf_p.tile([128, R, 2, D], BF16)
                    vm = vm_p.tile([128, R, 2, D + 1], BF16)

                qraw = qraw_p.tile([128, R * D], F32)
                kraw = kraw_p.tile([128, R * D], F32)
                vraw = vraw_p.tile([128, R * D], F32)
                nc.sync.dma_start(out=qraw, in_=pview(Qh, p))
                nc.sync.dma_start(out=kraw, in_=pview(Kh, p))
                # V on the Pool/SWDGE queue: a second concurrent input
                # stream in case HW parallelizes DMA queues across engines
                nc.gpsimd.dma_start(out=vraw, in_=pview(Vh, p))

                qrv = qraw.rearrange("q (r d) -> q r d", r=R)
                krv = kraw.rearrange("q (r d) -> q r d", r=R)
                vrv = vraw.rearrange("q (r d) -> q r d", r=R)

                qe = qe_p.tile([128, R * D], BF16)
                ke = ke_p.tile([128, R * D], BF16)
                qr2 = qr2_p.tile([128, R * D], BF16)
                kr2 = kr2_p.tile([128, R * D], BF16)
                qev = qe.rearrange("q (r d) -> q r d", r=R)
                kev = ke.rearrange("q (r d) -> q r d", r=R)
                qr2v = qr2.rearrange("q (r d) -> q r d", r=R)
                kr2v = kr2.rearrange("q (r d) -> q r d", r=R)

                # elu(x)+1 == min(exp(x),1) + relu(x)
                nc.scalar.activation(qe, qraw, AF.Exp)
                nc.scalar.activation(qr2, qraw, AF.Relu)
                nc.vector.scalar_tensor_tensor(
                    out=qf[:, :, u, :], in0=qev, scalar=1.0, in1=qr2v,
                    op0=ALU.min, op1=ALU.add)
                nc.scalar.activation(ke, kraw, AF.Exp)
                nc.scalar.activation(kr2, kraw, AF.Relu)
                nc.vector.scalar_tensor_tensor(
                    out=kf[:, :, u, :], in0=kev, scalar=1.0, in1=kr2v,
                    op0=ALU.min, op1=ALU.add)
                # vm[:,:,u,0:D] = V * mask (bf16), col D = mask
                nc.gpsimd.tensor_tensor(
                    out=vm[:, :, u, 0:D], in0=vrv,
                    in1=mtile[:, p, :].unsqueeze(-1).to_broadcast([128, R, D]),
                    op=ALU.mult)
                nc.gpsimd.tensor_copy(out=vm[:, :, u, D], in_=mtile[:, p, :])

                if u == 1:
                    if prev is not None:
                        tail(*prev)
                        prev = None
                    # KV+Ksum accumulation: [128,130]-wide, 32 steps
                    for r in range(R):
                        nc.tensor.matmul(kv, lhsT=kf[:, r], rhs=vm[:, r],
                                         start=(r == 0), stop=(r == R - 1))
                    # Qf^T batched 8-wide (full 2KB PSUM bank)
                    for b in range(R // 8):
                        tp = tp_ps.tile([128, 8, 128], BF16)
                        for j in range(8):
                            nc.tensor.transpose(tp[:, j], qf[:, 8 * b + j],
                                                identity)
                        if b % 2 == 0:
                            nc.scalar.activation(
                                qt[:, 8 * b:8 * b + 8, :], tp, AF.Copy)
                        else:
                            nc.vector.tensor_copy(
                                out=qt[:, 8 * b:8 * b + 8, :], in_=tp)

                    prev = (g, kv, qt)
            tail(*prev, last=True)
    nc.finalize()
    return nc


_NC_CACHE = None


def _get_nc():
    global _NC_CACHE
    if _NC_CACHE is None:
        _NC_CACHE = build_bass()
    return _NC_CACHE


def kernel(Q: np.ndarray, K: np.ndarray, V: np.ndarray, mask: np.ndarray,
           _trace: bool = False):
    B, H = 4, 16
    NP = B * H
    per = NP // N_CORES
    Qr = np.ascontiguousarray(np.asarray(Q, dtype=np.float32).reshape(NP, S, D))
    Kr = np.ascontiguousarray(np.asarray(K, dtype=np.float32).reshape(NP, S, D))
    Vr = np.ascontiguousarray(np.asarray(V, dtype=np.float32).reshape(NP, S, D))
    Mr = np.ascontiguousarray(np.asarray(mask, dtype=np.float32).reshape(NP, S))

    in_maps = []
    for i in range(N_CORES):
        sl = slice(i * per, (i + 1) * per)
        in_maps.append({
            "Q": np.ascontiguousarray(Qr[sl]),
            "K": np.ascontiguousarray(Kr[sl]),
            "V": np.ascontiguousarray(Vr[sl]),
            "mask": np.ascontiguousarray(Mr[sl]),
        })

    nc = _get_nc()
    res = run_bass_kernel_spmd(nc, in_maps, core_ids=list(range(N_CORES)),
                               trace=_trace)
    out = np.concatenate([r["O"] for r in res.results], axis=0)
    if _trace:
        kernel._last_results = res
    return out.reshape(B, H, S, D)


# revision 45
# speedup vs baseline: 1.0777x; 1.0777x over previous
"""Linear attention (elu(x)+1 feature map) Bass/Tile kernel for Trainium2.

Problem: B=4, H=16, S=4096, D=64, fp32.
  Qf = elu(Q)+1; Kf = (elu(K)+1)*mask
  KV = einsum('bhsd,bhse->bhde', Kf, V); Ksum = sum_s Kf
  out = (Qf @ KV) / (Qf . Ksum)

Sharding: the 64 (b,h) pairs are data-parallel; each of the 8 cores gets 8
pairs. No collectives.

Per-core design (v3; ~4x faster than the DMA-dispatch-bound v1):
  * "Fat-row" layout: s = 32*p + r (partition p holds 32 consecutive rows),
    so each pair's Q/K/V/O moves as ONE DMA of [128, 2048] with 8KB
    contiguous lines (vs 48 DMAs with 256B lines per pair before). ~29
    DMAs total; HBM transfer is the roofline (~34MB at ~360-400GB/s).
  * bf16 matmul operands (harness tolerance 2e-2; measured ~1.8e-3):
    PE runs at 1 cycle/row instead of 4 for fp32.
  * elu(x)+1 = min(exp(x),1) + relu(x) in 3 passes: exp + relu on ACT
    (bf16 out), then one fused DVE scalar_tensor_tensor
    (min(e,1) add r). GPSIMD tensor_scalar proved ~10x slow on real HW
    (software op), so relu stays on ACT.
  * mask folded into V during its bf16 conversion on Pool (V*m), and the
    raw mask appended as column 64 of the vm tile so the 130-wide KV
    accumulation matmul also yields Ksum = Kf^T m for free (both pairs
    of a group merged into one [128,130] matmul per 128-row step).
  * Qf^T via PE transposes batched 8-wide into one full PSUM bank,
    copied to SBUF alternately by ACT (Copy activation) and DVE.
  * Phase B: per r one [128,130] matmul against the block-diagonal
    [[KV_A|KsumA],0;0,[KV_B|KsumB]] gives out and denominator together;
    batches of 3 r per PSUM bank, then DVE reciprocal of the den columns
    (PSUM->SBUF; HW allows only one PSUM operand per DVE op) and one DVE
    multiply writes normalized output into the staging tile.
  * Software-pipelined across the 4 groups: group g-1's tail issues
    before group g's PE section; output DMAs go on the ACT queue (SP
    keeps the input stream), split in thirds to shorten the drain.

Benchmarked via reps-slope (K kernel bodies in one NEFF, dispatch
overhead cancels; pooled median of 24 interleaved samples): ~79us/exec
vs ~396us for the v1 baseline (~5x) — at the HBM roofline (25.2MB read
+ 8.4MB write per core, with reads/writes partially overlapping on HW).
"""

import numpy as np

import concourse.bass as bass
import concourse.mybir as mybir
import concourse.tile as tile
from concourse.bass_utils import run_bass_kernel_spmd
from concourse.masks import make_identity

F32 = mybir.dt.float32
BF16 = mybir.dt.bfloat16
AF = mybir.ActivationFunctionType
ALU = mybir.AluOpType

N_CORES = 8
PAIRS = 8          # (b,h) pairs per core
S = 4096
D = 64
R = 32             # rows per partition; s = 32*p + r
NGROUPS = PAIRS // 2


def build_bass(reps: int = 1) -> bass.Bass:
    from contextlib import ExitStack

    from concourse.bacc import Bacc
    nc = Bacc()
    Qh = nc.dram_tensor("Q", [PAIRS, S, D], F32, kind="ExternalInput")
    Kh = nc.dram_tensor("K", [PAIRS, S, D], F32, kind="ExternalInput")
    Vh = nc.dram_tensor("V", [PAIRS, S, D], F32, kind="ExternalInput")
    Mh = nc.dram_tensor("mask", [PAIRS, S], F32, kind="ExternalInput")
    Oh = nc.dram_tensor("O", [PAIRS, S, D], F32, kind="ExternalOutput")

    # per-pair fat-row views [128, 2048]; per-group output views [128,2,2048]
    def pview(h, p):
        return h[p].rearrange("(q r) d -> q (r d)", q=128, r=R)

    Mv = Mh.rearrange("u (q r) -> q u r", q=128, r=R)          # [128, 8, 32]
    Ov = [Oh[2 * g:2 * g + 2].rearrange("u (q r) d -> q u (r d)", q=128, r=R)
          for g in range(NGROUPS)]

    with tile.TileContext(nc) as tc:
        with ExitStack() as ctx:
            consts = ctx.enter_context(tc.tile_pool(name="consts", bufs=1))
            qraw_p = ctx.enter_context(tc.tile_pool(name="qraw", bufs=3))
            kraw_p = ctx.enter_context(tc.tile_pool(name="kraw", bufs=3))
            vraw_p = ctx.enter_context(tc.tile_pool(name="vraw", bufs=3))
            qe_p = ctx.enter_context(tc.tile_pool(name="qe", bufs=2))
            ke_p = ctx.enter_context(tc.tile_pool(name="ke", bufs=2))
            qr2_p = ctx.enter_context(tc.tile_pool(name="qr2", bufs=2))
            kr2_p = ctx.enter_context(tc.tile_pool(name="kr2", bufs=2))
            qf_p = ctx.enter_context(tc.tile_pool(name="qf", bufs=2))
            kf_p = ctx.enter_context(tc.tile_pool(name="kf", bufs=2))
            vm_p = ctx.enter_context(tc.tile_pool(name="vm", bufs=2))
            qt_p = ctx.enter_context(tc.tile_pool(name="qt", bufs=2))
            osb_p = ctx.enter_context(tc.tile_pool(name="osb", bufs=2))
            bd_p = ctx.enter_context(tc.tile_pool(name="bd", bufs=2))
            rec_p = ctx.enter_context(tc.tile_pool(name="rec", bufs=2))
            kv_ps = ctx.enter_context(
                tc.tile_pool(name="kvps", bufs=2, space="PSUM"))
            tp_ps = ctx.enter_context(
                tc.tile_pool(name="tpps", bufs=2, space="PSUM"))
            ob_ps = ctx.enter_context(
                tc.tile_pool(name="obps", bufs=4, space="PSUM"))

            identity = consts.tile([128, 128], BF16)
            make_identity(nc, identity)
            mtile = consts.tile([128, PAIRS, R], F32)
            nc.sync.dma_start(out=mtile, in_=Mv)

            def tail(g, kv, qt, last=False):
                g = g % NGROUPS
                """phaseB (out + den cols) / recip / normalize / output."""
                # bd = [[KV_A|KsumA], 0; 0, [KV_B|KsumB]] — one copy per pair
                bd = bd_p.tile([128, 2, D + 1], BF16)
                nc.vector.memset(bd, 0.0)
                nc.vector.tensor_copy(out=bd[0:64, 0, :], in_=kv[0:64, 0, :])
                nc.vector.tensor_copy(out=bd[64:128, 1, :],
                                      in_=kv[64:128, 1, :])

                osb = osb_p.tile([128, 2, R, D], F32)
                rec = rec_p.tile([128, R, 2], F32)
                # 32 r-steps in batches of 3 per PSUM bank (3*130 <= 512 f32)
                batches = [(s, min(3, R - s)) for s in range(0, R, 3)]
                # output DMA chunk boundaries (batch idx -> r range end)
                cuts = [2, 5, 8] if last else [4, 8]
                prev_r = [0]
                for bi, (r0, bsz) in enumerate(batches):
                    ob = ob_ps.tile([128, 3, 2, D + 1], F32)
                    for j in range(bsz):
                        nc.tensor.matmul(ob[:, j], lhsT=qt[:, r0 + j, :],
                                         rhs=bd, start=True, stop=True)
                    nc.vector.reciprocal(
                        rec[:, r0:r0 + bsz, :], ob[:, 0:bsz, :, D])
                    nc.vector.tensor_tensor(
                        out=osb[:, :, r0:r0 + bsz, :],
                        in0=ob[:, 0:bsz, :, 0:D].rearrange(
                            "q j u d -> q u j d"),
                        in1=rec[:, r0:r0 + bsz, :]
                        .rearrange("q j u -> q u j").unsqueeze(-1)
                        .to_broadcast([128, 2, bsz, D]),
                        op=ALU.mult)
                    if bi + 1 in cuts:
                        a, b2 = prev_r[0], r0 + bsz
                        nc.scalar.dma_start(
                            out=Ov[g][:, :, a * D:b2 * D],
                            in_=osb[:, :, a:b2, :].rearrange(
                                "q u r d -> q u (r d)"))
                        prev_r[0] = b2
                a = prev_r[0]
                nc.scalar.dma_start(
                    out=Ov[g][:, :, a * D:],
                    in_=osb[:, :, a:, :].rearrange("q u r d -> q u (r d)"))

            prev = None
            for p in range(PAIRS * reps):
                g, u = divmod(p % PAIRS, 2)
                g += (p // PAIRS) * NGROUPS
                p = p % PAIRS
                if u == 0:
                    kv = kv_ps.tile([128, 2, D + 1], F32)
                    qt = qt_p.tile([128, R, 128], BF16)
                    qf = qf_p.tile([128, R, 2, D], BF16)
                    kf = kf_p.tile([128, R, 2, D], BF16)
                    vm = vm_p.tile([128, R, 2, D + 1], BF16)

                qraw = qraw_p.tile([128, R * D], F32)
                kraw = kraw_p.tile([128, R * D], F32)
                vraw = vraw_p.tile([128, R * D], F32)
                halves = ([(0, R)] if p != PAIRS - 1 else
                          [(0, R // 2), (R // 2, R)])
                for (ra, rb) in halves:
                    sl = slice(ra * D, rb * D)
                    nc.sync.dma_start(out=qraw[:, sl], in_=pview(Qh, p)[:, sl])
                    nc.sync.dma_start(out=kraw[:, sl], in_=pview(Kh, p)[:, sl])
                    nc.sync.dma_start(out=vraw[:, sl], in_=pview(Vh, p)[:, sl])

                qrv = qraw.rearrange("q (r d) -> q r d", r=R)
                krv = kraw.rearrange("q (r d) -> q r d", r=R)
                vrv = vraw.rearrange("q (r d) -> q r d", r=R)

                qe = qe_p.tile([128, R * D], BF16)
                ke = ke_p.tile([128, R * D], BF16)
                qr2 = qr2_p.tile([128, R * D], BF16)
                kr2 = kr2_p.tile([128, R * D], BF16)
                qev = qe.rearrange("q (r d) -> q r d", r=R)
                kev = ke.rearrange("q (r d) -> q r d", r=R)
                qr2v = qr2.rearrange("q (r d) -> q r d", r=R)
                kr2v = kr2.rearrange("q (r d) -> q r d", r=R)

                # elu(x)+1 == min(exp(x),1) + relu(x); final pair runs in
                # two r-halves so its chain starts before the second half
                # of its inputs lands (kv matmuls have per-r deps)
                for (ra, rb) in halves:
                    sl = slice(ra * D, rb * D)
                    n = rb - ra
                    nc.scalar.activation(qe[:, sl], qraw[:, sl], AF.Exp)
                    nc.scalar.activation(qr2[:, sl], qraw[:, sl], AF.Relu)
                    nc.vector.scalar_tensor_tensor(
                        out=qf[:, ra:rb, u, :], in0=qev[:, ra:rb],
                        scalar=1.0, in1=qr2v[:, ra:rb],
                        op0=ALU.min, op1=ALU.add)
                    nc.scalar.activation(ke[:, sl], kraw[:, sl], AF.Exp)
                    nc.scalar.activation(kr2[:, sl], kraw[:, sl], AF.Relu)
                    nc.vector.scalar_tensor_tensor(
                        out=kf[:, ra:rb, u, :], in0=kev[:, ra:rb],
                        scalar=1.0, in1=kr2v[:, ra:rb],
                        op0=ALU.min, op1=ALU.add)
                    # vm[:,:,u,0:D] = V * mask (bf16), col D = mask
                    nc.gpsimd.tensor_tensor(
                        out=vm[:, ra:rb, u, 0:D], in0=vrv[:, ra:rb],
                        in1=mtile[:, p, ra:rb].unsqueeze(-1)
                        .to_broadcast([128, n, D]),
                        op=ALU.mult)
                    nc.gpsimd.tensor_copy(out=vm[:, ra:rb, u, D],
                                          in_=mtile[:, p, ra:rb])

                if u == 1:
                    if prev is not None:
                        tail(*prev)
                        prev = None
                    # KV+Ksum accumulation: [128,130]-wide, 32 steps
                    for r in range(R):
                        nc.tensor.matmul(kv, lhsT=kf[:, r], rhs=vm[:, r],
                                         start=(r == 0), stop=(r == R - 1))
                    # Qf^T batched 8-wide (full 2KB PSUM bank)
                    for b in range(R // 8):
                        tp = tp_ps.tile([128, 8, 128], BF16)
                        for j in range(8):
                            nc.tensor.transpose(tp[:, j], qf[:, 8 * b + j],
                                                identity)
                        if b % 2 == 0:
                            nc.scalar.activation(
                                qt[:, 8 * b:8 * b + 8, :], tp, AF.Copy)
                        else:
                            nc.vector.tensor_copy(
                                out=qt[:, 8 * b:8 * b + 8, :], in_=tp)

                    prev = (g, kv, qt)
            tail(*prev, last=True)
    nc.finalize()
    return nc


_NC_CACHE = None


def _get_nc():
    global _NC_CACHE
    if _NC_CACHE is None:
        _NC_CACHE = build_bass()
    return _NC_CACHE


def kernel(Q: np.ndarray, K: np.ndarray, V: np.ndarray, mask: np.ndarray,
           _trace: bool = False):
    B, H = 4, 16
    NP = B * H
    per = NP // N_CORES
    Qr = np.ascontiguousarray(np.asarray(Q, dtype=np.float32).reshape(NP, S, D))
    Kr = np.ascontiguousarray(np.asarray(K, dtype=np.float32).reshape(NP, S, D))
    Vr = np.ascontiguousarray(np.asarray(V, dtype=np.float32).reshape(NP, S, D))
    Mr = np.ascontiguousarray(np.asarray(mask, dtype=np.float32).reshape(NP, S))

    in_maps = []
    for i in range(N_CORES):
        sl = slice(i * per, (i + 1) * per)
        in_maps.append({
            "Q": np.ascontiguousarray(Qr[sl]),
            "K": np.ascontiguousarray(Kr[sl]),
            "V": np.ascontiguousarray(Vr[sl]),
            "mask": np.ascontiguousarray(Mr[sl]),
        })

    nc = _get_nc()
    res = run_bass_kernel_spmd(nc, in_maps, core_ids=list(range(N_CORES)),
                               trace=_trace)
    out = np.concatenate([r["O"] for r in res.results], axis=0)
    if _trace:
        kernel._last_results = res
    return out.reshape(B, H, S, D)
